# revision 33
# baseline (speedup 1.0000x reference)
"""Trainium2 Bass kernel for a single-head transformer encoder layer.

Problem shapes (hardcoded): B=4, S=4096, D=512, D_FFN=2048, fp32.
Sharding: 8 cores; core c handles batch b=c//2, query-row half h=c%2
(2048 q rows each). K/V for the batch's full sequence (4096 rows) are
projected on-core (duplicated across the 2 cores sharing a batch).

v2 structure (all matmuls float32r: 1 cycle/row, ~1.5e-4 rounding):
  pass 1: load q/k/v, PE-transpose to feature-major, project.
          QT [P,DC,M], KT [P,DC,S], V [P,S/P,D] stay resident in SBUF.
  pass 2: per 512-q block: scores S^T = lhsT(KT)@rhs(QT) -> exp on ACT
          (no max subtraction; scores ~ N(0,1)) -> P^T chunks feed the
          attn matmul (lhsT=PT, rhs=V) accumulating 32 chunks in PSUM;
          row sums ride along via a ones matmul. One drain per q block
          fused with 1/rsum, then +x, LN1; h rows spilled to DRAM.
  pass 3: FFN per 512-row block: re-read h, transpose, FFN1 (relu+bias
          fused in ACT copyback), FFN2, +b2 +h residual, LN2, store.
"""

import math
import threading
from contextlib import ExitStack

import numpy as np

import concourse.bass as bass
import concourse.tile as tile
from concourse import bacc, mybir
from concourse.bass_utils import run_bass_kernel_spmd
from concourse.masks import make_identity

P = 128
B, S, D = 4, 4096, 512
F = 4 * D                    # 2048
M = S // 2                   # q rows per core
DC = D // P                  # 4 feature chunks
FC = F // P                  # 16 ffn chunks
KB = 512                     # load-block rows
QB = 512                     # q-block cols
NQB = M // QB                # 4
SC = S // P                  # 32 k chunks
RC = M // P                  # 16 row chunks per core
EPS = 1e-5
SCALE = 1.0 / math.sqrt(D)
f32 = mybir.dt.float32
f32r = mybir.dt.float32r
N_CORES = 8


def _ln_stats(nc, pool, t):
    fmax = nc.vector.BN_STATS_FMAX
    if D <= fmax:
        stats = pool.tile([P, nc.vector.BN_STATS_DIM], f32, tag="ln_stats")
        nc.vector.bn_stats(out=stats[:], in_=t[:])
        mv = pool.tile([P, nc.vector.BN_AGGR_DIM], f32, tag="ln_mv")
        nc.vector.bn_aggr(out=mv[:], in_=stats[:])
    else:
        sub = math.gcd(fmax, D)
        nsub = D // sub
        tr = t.rearrange("p (n s) -> p n s", s=sub)
        stats = pool.tile([P, nsub, nc.vector.BN_STATS_DIM], f32, tag="ln_stats")
        for i in range(nsub):
            nc.vector.bn_stats(out=stats[:, i, :], in_=tr[:, i, :])
        mv = pool.tile([P, nc.vector.BN_AGGR_DIM], f32, tag="ln_mv")
        nc.vector.bn_aggr(out=mv[:], in_=stats[:])
    return mv[:, 0:1], mv[:, 1:2]


def _apply_ln(nc, pool, t, eps_t, gamma_bc, beta_bc):
    mean, var = _ln_stats(nc, pool, t)
    nc.scalar.activation(out=var, in_=var,
                         func=mybir.ActivationFunctionType.Sqrt,
                         bias=eps_t[:], scale=1.0, alpha=0.0)
    nc.vector.reciprocal(out=var, in_=var)
    nc.vector.tensor_scalar(out=t[:], in0=t[:], scalar1=mean, scalar2=var,
                            op0=mybir.AluOpType.subtract,
                            op1=mybir.AluOpType.mult)
    nc.vector.tensor_mul(out=t[:], in0=t[:], in1=gamma_bc[:])
    nc.vector.tensor_add(out=t[:], in0=t[:], in1=beta_bc[:])


def _bcast_load(nc, pool, vec_ap, n, tag):
    t = pool.tile([P, n], f32, tag=tag)
    src = bass.AP(tensor=vec_ap.tensor, offset=vec_ap.offset,
                  ap=[[0, P]] + list(vec_ap.ap))
    nc.gpsimd.dma_start(out=t[:], in_=src)
    return t


def _fm_load(nc, pool, vec_ap, chunks, tag):
    t = pool.tile([P, chunks], f32, tag=tag)
    nc.sync.dma_start(t[:], vec_ap.rearrange("(c p) -> p c", p=P))
    return t


def _load_w_fm(nc, pool, raw_pool, w_ap, kchunks, nout, tag):
    t = pool.tile([P, kchunks, nout], f32r, tag=tag)
    wr = w_ap.rearrange("(c p) n -> p c n", p=P)
    for c in range(kchunks):
        raw = raw_pool.tile([P, nout], f32, tag="w_raw")
        nc.sync.dma_start(raw[:], wr[:, c, :])
        nc.vector.tensor_copy(t[:, c, :], raw[:])
    return t


def _transpose_rows(nc, ps_pool, ident, nat, fm, rt):
    """PE-transpose nat [P,512] into fm[:, :, rt*P:(rt+1)*P] via one
    4-quadrant PSUM bank and a single batched DVE drain."""
    pst = ps_pool.tile([P, DC, P], f32, tag="ps_tp")
    for dc in range(DC):
        nc.tensor.transpose(pst[:, dc, :], nat[:, dc * P:(dc + 1) * P], ident)
    nc.vector.tensor_copy(fm[:, :, rt * P:(rt + 1) * P], pst[:])


def build_program():
    nc = bacc.Bacc()
    q = nc.dram_tensor("q", [M, D], f32, kind="ExternalInput")
    k = nc.dram_tensor("k", [S, D], f32, kind="ExternalInput")
    v = nc.dram_tensor("v", [S, D], f32, kind="ExternalInput")
    x = nc.dram_tensor("x", [M, D], f32, kind="ExternalInput")
    Wq = nc.dram_tensor("Wq", [D, D], f32, kind="ExternalInput")
    Wk = nc.dram_tensor("Wk", [D, D], f32, kind="ExternalInput")
    Wv = nc.dram_tensor("Wv", [D, D], f32, kind="ExternalInput")
    bq = nc.dram_tensor("bq", [D], f32, kind="ExternalInput")
    bk = nc.dram_tensor("bk", [D], f32, kind="ExternalInput")
    bv = nc.dram_tensor("bv", [D], f32, kind="ExternalInput")
    g1 = nc.dram_tensor("gamma1", [D], f32, kind="ExternalInput")
    be1 = nc.dram_tensor("beta1", [D], f32, kind="ExternalInput")
    W1 = nc.dram_tensor("W1", [D, F], f32, kind="ExternalInput")
    b1 = nc.dram_tensor("b1", [F], f32, kind="ExternalInput")
    W2 = nc.dram_tensor("W2", [F, D], f32, kind="ExternalInput")
    b2 = nc.dram_tensor("b2", [D], f32, kind="ExternalInput")
    g2 = nc.dram_tensor("gamma2", [D], f32, kind="ExternalInput")
    be2 = nc.dram_tensor("beta2", [D], f32, kind="ExternalInput")
    out = nc.dram_tensor("out", [M, D], f32, kind="ExternalOutput")
    h_dram = nc.dram_tensor("h_scratch", [M, D], f32)

    with tile.TileContext(nc) as tc, ExitStack() as ctx:
        g_pool = ctx.enter_context(tc.tile_pool(name="glob", bufs=1))
        io = ctx.enter_context(tc.tile_pool(name="io", bufs=3))
        ep = ctx.enter_context(tc.tile_pool(name="ep", bufs=2))
        ps_a = ctx.enter_context(tc.tile_pool(name="ps_a", bufs=2, space="PSUM"))

        ident_t = g_pool.tile([P, P], f32, tag="ident")
        make_identity(nc, ident_t[:])
        ident = ident_t[:]
        ones32 = g_pool.tile([P, 4], f32, tag="ones32")
        nc.vector.memset(ones32[:], 1.0)
        ones_r = g_pool.tile([P, 4], f32r, tag="ones")
        nc.vector.tensor_copy(ones_r[:], ones32[:])
        eps_t = g_pool.tile([P, 1], f32, tag="eps")
        nc.vector.memset(eps_t[:], EPS)

        with ExitStack() as actx:
            attn = actx.enter_context(tc.tile_pool(name="attn", bufs=1))
            qt_full = attn.tile([P, DC, M], f32r, tag="qt_full")
            kt_full = attn.tile([P, DC, S], f32r, tag="kt_full")
            v_full = attn.tile([P, SC, D], f32r, tag="v_full")

            # ---------- pass 1: load + transpose + project ----------
            with ExitStack() as p1ctx:
                p1 = p1ctx.enter_context(tc.tile_pool(name="ph1", bufs=1))
                p1r = p1ctx.enter_context(tc.tile_pool(name="ph1r", bufs=1))
                tp = p1ctx.enter_context(tc.tile_pool(name="tp", bufs=1))
                ps_p = p1ctx.enter_context(
                    tc.tile_pool(name="ps_p", bufs=3, space="PSUM"))

                bq_fm = _fm_load(nc, p1, bq[:], DC, "bq")
                bk_fm = _fm_load(nc, p1, bk[:], DC, "bk")
                bv_bc = _bcast_load(nc, p1, bv[:], D, "bv")

                # q and k: feature-major projections (lhsT = W chunk)
                for name, src, rows, w_ap, b_fm, dst in (
                        ("q", q, M, Wq, None, qt_full),
                        ("k", k, S, Wk, None, kt_full)):
                    w_sb = _load_w_fm(nc, p1, p1r, w_ap[:], DC, D, "w_sb")
                    b_fm = bq_fm if name == "q" else bk_fm
                    for j in range(rows // KB):
                        fmr = tp.tile([P, DC, KB], f32r, tag="in_fm",
                                      name=name + "_fm")
                        for rt in range(KB // P):
                            nat = io.tile([P, D], f32, tag="in_nat",
                                          name=name + "_nat")
                            nc.sync.dma_start(
                                nat[:], src[j * KB + rt * P:
                                            j * KB + (rt + 1) * P, :])
                            _transpose_rows(nc, ps_a, ident, nat[:], fmr, rt)
                        for m in range(DC):
                            psp = ps_p.tile([P, KB], f32, tag="ps_proj")
                            for kc in range(DC):
                                nc.tensor.matmul(
                                    psp[:],
                                    lhsT=w_sb[:, kc, m * P:(m + 1) * P],
                                    rhs=fmr[:, kc, :],
                                    start=(kc == 0), stop=(kc == DC - 1))
                            nc.vector.tensor_scalar_add(
                                out=dst[:, m, j * KB:(j + 1) * KB],
                                in0=psp[:], scalar1=b_fm[:, m:m + 1])

                # v: natural projection (lhsT = vT chunk, rhs = Wv)
                wv_sb = _load_w_fm(nc, p1, p1r, Wv[:], DC, D, "w_sb")
                for j in range(S // KB):
                    fmr = tp.tile([P, DC, KB], f32r, tag="in_fm", name="v_fm")
                    for rt in range(KB // P):
                        nat = io.tile([P, D], f32, tag="in_nat", name="v_nat")
                        nc.sync.dma_start(
                            nat[:], v[j * KB + rt * P:j * KB + (rt + 1) * P, :])
                        _transpose_rows(nc, ps_a, ident, nat[:], fmr, rt)
                    for rt in range(KB // P):
                        psv = ps_p.tile([P, D], f32, tag="ps_proj")
                        for kc in range(DC):
                            nc.tensor.matmul(
                                psv[:], lhsT=fmr[:, kc, rt * P:(rt + 1) * P],
                                rhs=wv_sb[:, kc, :],
                                start=(kc == 0), stop=(kc == DC - 1))
                        nc.vector.tensor_add(
                            out=v_full[:, j * (KB // P) + rt, :],
                            in0=psv[:], in1=bv_bc[:])

            # ---------- pass 2: attention + LN1, h -> DRAM ----------
            with ExitStack() as p2ctx:
                p2 = p2ctx.enter_context(tc.tile_pool(name="ph2", bufs=1))
                ptp = p2ctx.enter_context(tc.tile_pool(name="ptp", bufs=4))
                ps_o = p2ctx.enter_context(
                    tc.tile_pool(name="ps_o", bufs=4, space="PSUM"))
                ps_r = p2ctx.enter_context(
                    tc.tile_pool(name="ps_r", bufs=2, space="PSUM"))

                g1_bc = _bcast_load(nc, p2, g1[:], D, "g1")
                be1_bc = _bcast_load(nc, p2, be1[:], D, "be1")

                for qb in range(NQB):
                    po = [ps_o.tile([P, D], f32, tag="ps_out", name=f"po{i}")
                          for i in range(4)]
                    rsum_sb = ep.tile([P, 4], f32, tag="rsum_sb")
                    for kc in range(SC):
                        pss = ps_a.tile([P, QB], f32, tag="ps_tp", name="pss")
                        for dc in range(DC):
                            nc.tensor.matmul(
                                pss[:], lhsT=kt_full[:, dc, kc * P:(kc + 1) * P],
                                rhs=qt_full[:, dc, qb * QB:(qb + 1) * QB],
                                start=(dc == 0), stop=(dc == DC - 1))
                        ptile = ptp.tile([P, QB], f32r, tag="pt")
                        nc.scalar.activation(
                            out=ptile[:], in_=pss[:],
                            func=mybir.ActivationFunctionType.Exp,
                            bias=0.0, scale=SCALE, alpha=0.0)
                        pr = ps_r.tile([P, 4, 4], f32, tag="ps_rsum")
                        for qc in range(4):
                            nc.tensor.matmul(
                                po[qc][:], lhsT=ptile[:, qc * P:(qc + 1) * P],
                                rhs=v_full[:, kc, :],
                                start=(kc == 0), stop=(kc == SC - 1))
                            nc.tensor.matmul(
                                pr[:, qc, :],
                                lhsT=ptile[:, qc * P:(qc + 1) * P],
                                rhs=ones_r[:],
                                start=True, stop=True)
                        if kc == 0:
                            nc.vector.tensor_copy(rsum_sb[:], pr[:, :, 0])
                        else:
                            nc.vector.tensor_add(out=rsum_sb[:],
                                                 in0=rsum_sb[:],
                                                 in1=pr[:, :, 0])
                    rinv = ep.tile([P, 4], f32, tag="rinv")
                    nc.vector.reciprocal(out=rinv[:], in_=rsum_sb[:])
                    for qc in range(4):
                        rc = qb * 4 + qc
                        t = ep.tile([P, D], f32, tag="row_t", name="hrow")
                        nc.vector.tensor_scalar_mul(
                            out=t[:], in0=po[qc][:],
                            scalar1=rinv[:, qc:qc + 1])
                        xt = io.tile([P, D], f32, tag="in_nat", name="x_nat")
                        nc.sync.dma_start(xt[:], x[rc * P:(rc + 1) * P, :])
                        nc.vector.tensor_add(out=t[:], in0=t[:], in1=xt[:])
                        _apply_ln(nc, ep, t[:], eps_t, g1_bc, be1_bc)
                        nc.sync.dma_start(h_dram[rc * P:(rc + 1) * P, :], t[:])

        # ---------- pass 3: FFN + LN2 ----------
        with ExitStack() as p3ctx:
            p3 = p3ctx.enter_context(tc.tile_pool(name="ph3", bufs=1))
            hp = p3ctx.enter_context(tc.tile_pool(name="hp", bufs=4))
            htp = p3ctx.enter_context(tc.tile_pool(name="htp", bufs=2))
            p3r = p3ctx.enter_context(tc.tile_pool(name="ph3r", bufs=2))
            f1p = p3ctx.enter_context(tc.tile_pool(name="f1p", bufs=2))
            ps_f = p3ctx.enter_context(
                tc.tile_pool(name="ps_f", bufs=2, space="PSUM"))
            ps_g = p3ctx.enter_context(
                tc.tile_pool(name="ps_g", bufs=2, space="PSUM"))

            g2_bc = _bcast_load(nc, p3, g2[:], D, "g2")
            be2_bc = _bcast_load(nc, p3, be2[:], D, "be2")
            b2_bc = _bcast_load(nc, p3, b2[:], D, "b2")
            b1_fm = _fm_load(nc, p3, b1[:], FC, "b1")
            w1_sb = _load_w_fm(nc, p3, p3r, W1[:], DC, F, "w1")
            w2_sb = _load_w_fm(nc, p3, p3r, W2[:], FC, D, "w2")

            for fb in range(NQB):
                hrows = []
                htr = htp.tile([P, DC, QB], f32r, tag="ht_blk")
                for qc in range(4):
                    rc = fb * 4 + qc
                    hrow = hp.tile([P, D], f32, tag="h_nat", name=f"h{qc}")
                    nc.sync.dma_start(hrow[:],
                                      h_dram[rc * P:(rc + 1) * P, :])
                    hrows.append(hrow)
                    _transpose_rows(nc, ps_a, ident, hrow[:], htr, qc)
                f1t = f1p.tile([P, FC, QB], f32r, tag="f1t")
                for fc in range(FC):
                    psf = ps_f.tile([P, QB], f32, tag="ps_ffn")
                    for dc in range(DC):
                        nc.tensor.matmul(
                            psf[:], lhsT=w1_sb[:, dc, fc * P:(fc + 1) * P],
                            rhs=htr[:, dc, :],
                            start=(dc == 0), stop=(dc == DC - 1))
                    nc.scalar.activation(
                        out=f1t[:, fc, :], in_=psf[:],
                        func=mybir.ActivationFunctionType.Relu,
                        bias=b1_fm[:, fc:fc + 1], scale=1.0, alpha=0.0)
                for qc in range(4):
                    rc = fb * 4 + qc
                    pso = ps_g.tile([P, D], f32, tag="ps_out2")
                    for fc in range(FC):
                        nc.tensor.matmul(
                            pso[:], lhsT=f1t[:, fc, qc * P:(qc + 1) * P],
                            rhs=w2_sb[:, fc, :],
                            start=(fc == 0), stop=(fc == FC - 1))
                    t = ep.tile([P, D], f32, tag="row_t", name="out_t")
                    nc.vector.tensor_add(out=t[:], in0=pso[:], in1=b2_bc[:])
                    nc.vector.tensor_add(out=t[:], in0=t[:],
                                         in1=hrows[qc][:])
                    _apply_ln(nc, ep, t[:], eps_t, g2_bc, be2_bc)
                    nc.sync.dma_start(out[rc * P:(rc + 1) * P, :], t[:])

    nc.finalize()
    return nc


_CACHE = {}
_LOCK = threading.Lock()


def _get_program():
    with _LOCK:
        if "nc" not in _CACHE:
            _CACHE["nc"] = build_program()
        return _CACHE["nc"]


def kernel(**inputs):
    nc = _get_program()
    weights = {n: np.ascontiguousarray(inputs[n]) for n in
               ["Wq", "bq", "Wk", "bk", "Wv", "bv", "gamma1", "beta1",
                "W1", "b1", "W2", "b2", "gamma2", "beta2"]}
    in_maps = []
    for c in range(N_CORES):
        b, h = c // 2, c % 2
        sl = slice(h * M, (h + 1) * M)
        in_maps.append({
            "q": np.ascontiguousarray(inputs["q"][b, sl]),
            "k": np.ascontiguousarray(inputs["k"][b]),
            "v": np.ascontiguousarray(inputs["v"][b]),
            "x": np.ascontiguousarray(inputs["x"][b, sl]),
            **weights,
        })
    res = run_bass_kernel_spmd(nc, in_maps, list(range(N_CORES)))
    out = np.empty((B, S, D), np.float32)
    for c in range(N_CORES):
        b, h = c // 2, c % 2
        out[b, h * M:(h + 1) * M] = res.results[c]["out"]
    return out
